# revision 11
# baseline (speedup 1.0000x reference)
"""Trainium2 Bass kernel for windowed embedding lookup (nn_AttentionLayer).

Computation:
  out[b,s,e] = sum_k w[k,e] * data[snip_b, clip(inputs[b,s]+k-5, 0, 165), 0, e]

Strategy (data-parallel over batch, 2 batches per core on 8 cores):
  1. The host stages, per core, the two snippets' clip-padded table
     slices in transposed [e,p] bf16 layout with both batches
     interleaved per e-chunk, the diagonal weight blocks
     diag(w[k, e-chunk]) (bf16, identity prepended), and a sorted
     one-hot gather matrix (1126 real slots per batch, no padding);
     host work is layout/indexing only.
  2. The 11-tap conv runs per e-chunk on the TensorEngine in [e,p]
     orientation with BOTH batches in one rhs stream (332 cols/tap):
     11 PSUM-accumulated matmuls per chunk with the diag block
     stationary, halving LDWEIGHTS count vs per-batch taps.  Four
     transpose matmuls per chunk then produce the position-window
     views CA = C[0:128], CB = C[38:166] for both batches into a
     paired bf16 PSUM tile drained per chunk.
  3. Because out[s] = C[idx_s], the gather is a one-hot matmul over
     sorted indices (tiles 0..5 hit CA, 6..8 hit CB; asserted
     host-side): 9 tiles x (512+256)-col matmuls per batch (last tile
     102 rows), 4-deep PSUM after the conv pools release.  PSUM
     drains to bf16 alternate DVE/ACT; out rows DMA in per-batch
     pairs, the final tile split across both engines and both HWDGE
     rings.  The host un-sorts rows and casts to f32.
"""

import sys

for _p in ("/opt/trn_rl_repo",):
    if _p not in sys.path:
        sys.path.insert(0, _p)

import numpy as np

N_CORES = 8
B = 16
BPC = B // N_CORES  # batches per core
S = 1126
E = 768
EC = 6  # number of 128-wide e chunks
P = 166  # table positions
PPAD = 176  # padded positions (5 on each side)
W = 11
NSNIP = 100
NTILES = 9  # gather tiles per batch (sorted); last tile is 102 wide
LASTW = S - (NTILES - 1) * 128  # 102
NT_A = 6  # tiles 0..5 gather from CA (rows 0..127)
CB_BASE = 38  # CB covers table rows 38..165
NBLK = EC * W  # 66 diag blocks

_cache = {}


def _build(debug=False):
    import concourse.mybir as mybir
    import concourse.tile as tile
    from concourse import bacc

    f32 = mybir.dt.float32
    bf16 = mybir.dt.bfloat16

    nc = bacc.Bacc()

    # per-core snippet slices, both batches interleaved per chunk:
    #   col (c*2+b)*176 + q -> data[snip_b, clip(q-5), 0, c*128+i]
    tab2 = nc.declare_dram_parameter(
        "tab2", [128, EC * BPC * PPAD], bf16, isOutput=False
    )
    # block 0 = identity; block 1+c*11+k = diag(w[k, c-chunk]):
    #   [i, (1+c*11+k)*128 + j] = w[k, c*128+i] iff i==j
    diagw = nc.declare_dram_parameter(
        "diagw", [128, (NBLK + 1) * 128], bf16, isOutput=False
    )
    # host-built one-hot: [p, b*S + j] = 1 iff p == loc(b, j)
    ohh = nc.declare_dram_parameter("ohh", [128, BPC * S], bf16, isOutput=False)
    out = nc.declare_dram_parameter("out", [BPC * S, E], bf16, isOutput=True)

    with tile.TileContext(nc) as tc:
        with (
            tc.tile_pool(name="const", bufs=1) as constp,
            tc.tile_pool(name="ct", bufs=3) as ctp,
            tc.tile_pool(name="ob", bufs=6) as obp,
        ):
            # 2 gather banks live from the start (fills the 8-bank budget
            # alongside the conv pools) so the first gathers don't wait on
            # the conv-pool release barrier; 2 more banks after release.
            # Allocated first: pool releases must be LIFO.
            psg1 = tc.alloc_tile_pool(name="psum_g1", bufs=2, space="PSUM")
            psumt = tc.alloc_tile_pool(name="psum_t", bufs=2, space="PSUM")
            psumw = tc.alloc_tile_pool(name="psum_w", bufs=2, space="PSUM")

            t2m = constp.tile([128, EC, BPC, PPAD], bf16, tag="t2m")
            diagb = constp.tile([128, NBLK + 1, 128], bf16, tag="diagb")
            oht = constp.tile([128, BPC, S], bf16, tag="oht")
            win = constp.tile([128, BPC, 2, E], bf16, tag="win")
            identt = diagb[:, 0, :]

            # ---- input DMAs: each diag chunk split across BOTH HWDGE
            # rings so arrival paces the merged conv's consumption rate
            # (442KB per 1.6us chunk); small lead pieces cut the latency
            # to the first tap; one-hot halves late (needed at gathers)
            def diag_piece(eng, b0, b1):
                eng.dma_start(
                    out=diagb[:, b0:b1, :],
                    in_=diagw[:, b0 * 128 : b1 * 128].rearrange(
                        "p (k j) -> p k j", j=128
                    ),
                )

            CW = BPC * PPAD  # tab2 cols per chunk

            def t2_piece(eng, c0, c1):
                eng.dma_start(
                    out=t2m[:, c0:c1, :, :].rearrange("p c b q -> p (c b q)"),
                    in_=tab2[:, c0 * CW : c1 * CW],
                )

            def ohh_piece(eng, b):
                eng.dma_start(
                    out=oht[:, b, :], in_=ohh[:, b * S : (b + 1) * S]
                )

            diag_piece(nc.sync, 0, 3)  # identity + chunk-0 taps 0-1
            t2_piece(nc.scalar, 0, 1)
            diag_piece(nc.sync, 3, 9)  # chunk-0 taps 2-7
            diag_piece(nc.scalar, 9, 12)  # chunk-0 taps 8-10
            diag_piece(nc.sync, 12, 18)  # chunk-1 lo
            t2_piece(nc.scalar, 1, 2)
            diag_piece(nc.sync, 23, 29)  # chunk-2 lo
            diag_piece(nc.scalar, 18, 23)  # chunk-1 hi
            diag_piece(nc.sync, 34, 40)  # chunk-3 lo
            t2_piece(nc.scalar, 2, 3)
            diag_piece(nc.sync, 45, 51)  # chunk-4 lo
            diag_piece(nc.scalar, 29, 34)  # chunk-2 hi
            diag_piece(nc.sync, 56, 62)  # chunk-5 lo
            t2_piece(nc.scalar, 3, 4)
            diag_piece(nc.sync, 51, 56)  # chunk-4 hi
            diag_piece(nc.scalar, 40, 45)  # chunk-3 hi
            diag_piece(nc.sync, 62, 67)  # chunk-5 hi
            t2_piece(nc.scalar, 4, 5)
            ohh_piece(nc.sync, 0)
            t2_piece(nc.scalar, 5, 6)
            ohh_piece(nc.scalar, 1)

            dr = [0]
            dengines = (nc.vector.tensor_copy, nc.scalar.copy)

            def drain(dst, src):
                dengines[dr[0] % 2](dst, src)
                dr[0] += 1

            cts = {}

            def conv_taps(c):
                # conv in [e,p]: stationary diag block, both batches streamed
                pT = psumt.tile([128, BPC, P], f32, tag="pT", name="pT")
                for k in range(W):
                    nc.tensor.matmul(
                        out=pT[:, :, :],
                        lhsT=diagb[:, 1 + c * W + k, :],
                        rhs=t2m[:, c, :, k : k + P],
                        start=(k == 0),
                        stop=(k == W - 1),
                    )
                ct = ctp.tile([128, BPC, P], bf16, tag="ct", name="ct")
                drain(ct[:, :, :], pT[:, :, :])
                cts[c] = ct

            def conv_tp(c):
                # both windows of both batches into one paired bf16 PSUM
                # tile, drained immediately into the window tile
                cw = psumw.tile([128, BPC * 2, 128], bf16, tag="cw", name="cw")
                for b in range(BPC):
                    nc.tensor.transpose(
                        out=cw[:, b * 2, :],
                        in_=cts[c][:, b, 0:128],
                        identity=identt,
                    )
                    nc.tensor.transpose(
                        out=cw[:, b * 2 + 1, :],
                        in_=cts[c][:, b, CB_BASE : CB_BASE + 128],
                        identity=identt,
                    )
                drain(
                    win[:, :, :, c * 128 : (c + 1) * 128],
                    cw[:, :, :].rearrange("p (b w) j -> p b w j", w=2),
                )

            # ---- conv: weave transposes one chunk behind the taps so the
            # PE never waits on a ct drain
            conv_taps(0)
            conv_taps(1)
            conv_tp(0)
            for c in range(2, EC):
                conv_taps(c)
                conv_tp(c - 1)
            conv_tp(EC - 1)

            # conv PSUM done: release while the first gathers run on psg1
            psumw.release()
            psumt.release()
            psg2 = tc.alloc_tile_pool(name="psum_g2", bufs=2, space="PSUM")
            gpools = [psg1, psg2]
            gi = [0]

            obcur = [None]

            def pdrain(dst, src):
                # gpsimd cannot read PSUM on TRN2: DVE/ACT alternate
                drain(dst, src)

            def gather_tile(b, t, last=False):
                # single-pass gather: out[j, e] = sum_p oh[p, j] * C[p, e]
                tw = LASTW if t == NTILES - 1 else 128
                cc = win[:, b, 0, :] if t < NT_A else win[:, b, 1, :]
                pool = gpools[gi[0] % 2]
                gi[0] += 1
                pso = pool.tile([128, E], f32, tag="po", name="pso")
                for n0, nw in ((0, 512), (512, 256)):
                    nc.tensor.matmul(
                        out=pso[0:tw, n0 : n0 + nw],
                        lhsT=oht[:, b, t * 128 : t * 128 + tw],
                        rhs=cc[:, n0 : n0 + nw],
                        start=True,
                        stop=True,
                    )
                if t % 2 == 0:
                    obcur[0] = obp.tile([128, 2, E], bf16, tag="ob", name="ob2")
                ob2 = obcur[0]
                r0 = b * S + t * 128
                if last:
                    # final tile: drain halves on both engines, DMA halves on
                    # both HWDGE rings so the completions overlap
                    nc.vector.tensor_copy(ob2[0:tw, 0, 0:384], pso[0:tw, 0:384])
                    nc.scalar.copy(ob2[0:tw, 0, 384:768], pso[0:tw, 384:768])
                    nc.sync.dma_start(
                        out=out[r0 : r0 + tw, 0:384], in_=ob2[0:tw, 0, 0:384]
                    )
                    nc.scalar.dma_start(
                        out=out[r0 : r0 + tw, 384:768],
                        in_=ob2[0:tw, 0, 384:768],
                    )
                    return
                pdrain(ob2[0:tw, t % 2, :], pso[0:tw, :])
                if t % 2 == 1:
                    nc.sync.dma_start(
                        out=out[r0 - 128 : r0 + 128, :].rearrange(
                            "(t p) e -> p t e", t=2
                        ),
                        in_=ob2[:, 0:2, :],
                    )
                elif t == NTILES - 1:
                    nc.sync.dma_start(
                        out=out[r0 : r0 + tw, :], in_=ob2[0:tw, 0, :]
                    )

            for b in range(BPC):
                for t in range(NTILES):
                    gather_tile(b, t, last=(b == BPC - 1 and t == NTILES - 1))
            psg2.release()
            psg1.release()

    nc.finalize()
    return nc


def _get_nc():
    if "nc" not in _cache:
        _cache["nc"] = _build()
    return _cache["nc"]


def _prep_shared(data, w):
    # layout-only host staging (no arithmetic)
    import ml_dtypes

    bf = ml_dtypes.bfloat16
    d0 = np.asarray(data, dtype=np.float32)[:, :, 0, :]  # [100, 166, 768]
    # clip-pad positions to [176]
    dp = np.concatenate(
        [np.repeat(d0[:, :1], 5, axis=1), d0, np.repeat(d0[:, -1:], 5, axis=1)],
        axis=1,
    )  # [100, 176, 768]
    dT = np.transpose(dp, (0, 2, 1))  # [100, 768, 176]
    dT = dT.reshape(NSNIP, EC, 128, PPAD).transpose(0, 2, 1, 3)
    tabs = np.ascontiguousarray(dT.astype(bf))  # [100, 128, EC, PPAD]

    wT = np.asarray(w, dtype=np.float32).T  # [768, 11]
    w2 = wT.reshape(EC, 128, W).transpose(1, 0, 2).reshape(128, NBLK)
    diagw = np.zeros((128, NBLK + 1, 128), dtype=bf)
    ii = np.arange(128)
    diagw[ii, 0, ii] = 1  # block 0 = identity (for transpose matmuls)
    diagw[ii, 1:, ii] = w2.astype(bf)
    diagw = np.ascontiguousarray(diagw.reshape(128, (NBLK + 1) * 128))
    return tabs, diagw


def _prep_batch(idx_row):
    """Sort one batch's indices; return (one-hot [128, S] bf16, rank)."""
    import ml_dtypes

    v = np.asarray(idx_row, dtype=np.int64)
    order = np.argsort(v, kind="stable")
    vs = v[order]
    # sorted tiles 0..5 must fit CA rows [0,127]; tiles 6..8 CB rows [38,165]
    assert vs[NT_A * 128 - 1] <= 127, "gather tile/window layout violated (A)"
    assert vs[NT_A * 128] >= CB_BASE, "gather tile/window layout violated (B)"
    base = np.repeat([0] * NT_A + [CB_BASE] * (NTILES - NT_A), 128)[:S]
    loc = vs - base
    assert loc.min() >= 0 and loc.max() < 128
    oh = np.zeros((128, S), dtype=ml_dtypes.bfloat16)
    oh[loc, np.arange(S)] = 1
    rank = np.empty(S, dtype=np.int64)
    rank[order] = np.arange(S)
    return oh, rank


def kernel(inputs, code_snippet_id, data, w, _trace=False):
    from concourse.bass_utils import run_bass_kernel_spmd

    nc = _get_nc()
    inputs = np.asarray(inputs, dtype=np.int32)
    snips = np.asarray(code_snippet_id, dtype=np.int32).reshape(-1)
    tabs, diagw = _prep_shared(data, w)

    in_maps = []
    ranks = []
    for ci in range(N_CORES):
        b0 = ci * BPC
        ohs = []
        for b in range(BPC):
            oh, rank = _prep_batch(inputs[b0 + b])
            ohs.append(oh)
            ranks.append(rank)
        tb = np.stack([tabs[snips[b0 + b]] for b in range(BPC)], axis=2)
        in_maps.append(
            {
                "tab2": np.ascontiguousarray(tb.reshape(128, EC * BPC * PPAD)),
                "diagw": diagw,
                "ohh": np.ascontiguousarray(np.concatenate(ohs, axis=1)),
            }
        )

    res = run_bass_kernel_spmd(
        nc, in_maps, core_ids=list(range(N_CORES)), trace=_trace
    )
    _cache["last_results"] = res
    outs = []
    for ci in range(N_CORES):
        o = np.asarray(res.results[ci]["out"]).reshape(BPC, S, E)
        for b in range(BPC):
            outs.append(o[b, ranks[ci * BPC + b]].astype(np.float32))
    return np.stack(outs, axis=0)


# revision 12
# speedup vs baseline: 1.1331x; 1.1331x over previous
"""Trainium2 Bass kernel for windowed embedding lookup (nn_AttentionLayer).

Computation:
  out[b,s,e] = sum_k w[k,e] * data[snip_b, clip(inputs[b,s]+k-5, 0, 165), 0, e]

Strategy (data-parallel over batch, 2 batches per core on 8 cores):
  1. The host stages, per core, the two snippets' clip-padded table
     slices in transposed [e,p] bf16 layout with both batches
     interleaved per e-chunk, the diagonal weight blocks
     diag(w[k, e-chunk]) (bf16, identity prepended), and a sorted
     one-hot gather matrix (1126 real slots per batch, no padding);
     host work is layout/indexing only.
  2. The 11-tap conv runs per e-chunk on the TensorEngine in [e,p]
     orientation with BOTH batches in one rhs stream (332 cols/tap):
     11 PSUM-accumulated matmuls per chunk with the diag block
     stationary, halving LDWEIGHTS count vs per-batch taps.  Four
     transpose matmuls per chunk then produce the position-window
     views CA = C[0:128], CB = C[38:166] for both batches into a
     paired bf16 PSUM tile drained per chunk.
  3. Because out[s] = C[idx_s], the gather is a one-hot matmul over
     sorted indices (tiles 0..5 hit CA, 6..8 hit CB; asserted
     host-side): 9 tiles x (512+256)-col matmuls per batch (last tile
     102 rows), 4-deep PSUM after the conv pools release.  PSUM
     drains to bf16 alternate DVE/ACT; out rows DMA in per-batch
     pairs, the final tile split across both engines and both HWDGE
     rings.  The host un-sorts rows and casts to f32.
"""

import sys

for _p in ("/opt/trn_rl_repo",):
    if _p not in sys.path:
        sys.path.insert(0, _p)

import numpy as np

N_CORES = 8
B = 16
BPC = B // N_CORES  # batches per core
S = 1126
E = 768
EC = 6  # number of 128-wide e chunks
P = 166  # table positions
PPAD = 176  # padded positions (5 on each side)
W = 11
NSNIP = 100
NTILES = 9  # gather tiles per batch (sorted); last tile is 102 wide
LASTW = S - (NTILES - 1) * 128  # 102
NT_A = 6  # tiles 0..5 gather from CA (rows 0..127)
CB_BASE = 38  # CB covers table rows 38..165
NBLK = EC * W  # 66 diag blocks

_cache = {}


def _build(debug=False):
    import concourse.mybir as mybir
    import concourse.tile as tile
    from concourse import bacc

    f32 = mybir.dt.float32
    bf16 = mybir.dt.bfloat16

    nc = bacc.Bacc()

    # per-core snippet slices, both batches interleaved per chunk:
    #   col (c*2+b)*176 + q -> data[snip_b, clip(q-5), 0, c*128+i]
    tab2 = nc.declare_dram_parameter(
        "tab2", [128, EC * BPC * PPAD], bf16, isOutput=False
    )
    # block 0 = identity; block 1+c*11+k = diag(w[k, c-chunk]):
    #   [i, (1+c*11+k)*128 + j] = w[k, c*128+i] iff i==j
    diagw = nc.declare_dram_parameter(
        "diagw", [128, (NBLK + 1) * 128], bf16, isOutput=False
    )
    # host-built one-hot: [p, b*S + j] = 1 iff p == loc(b, j)
    ohh = nc.declare_dram_parameter("ohh", [128, BPC * S], bf16, isOutput=False)
    out = nc.declare_dram_parameter("out", [BPC * S, E], bf16, isOutput=True)

    with tile.TileContext(nc) as tc:
        with (
            tc.tile_pool(name="const", bufs=1) as constp,
            tc.tile_pool(name="ct", bufs=3) as ctp,
            tc.tile_pool(name="ob", bufs=6) as obp,
        ):
            # 2 gather banks live from the start (fills the 8-bank budget
            # alongside the conv pools) so the first gathers don't wait on
            # the conv-pool release barrier; 2 more banks after release.
            # Allocated first: pool releases must be LIFO.
            psg1 = tc.alloc_tile_pool(name="psum_g1", bufs=2, space="PSUM")
            psumt = tc.alloc_tile_pool(name="psum_t", bufs=2, space="PSUM")
            psumw = tc.alloc_tile_pool(name="psum_w", bufs=2, space="PSUM")

            t2m = constp.tile([128, EC, BPC, PPAD], bf16, tag="t2m")
            diagb = constp.tile([128, NBLK + 1, 128], bf16, tag="diagb")
            oht = constp.tile([128, BPC, S], bf16, tag="oht")
            win = constp.tile([128, BPC, 2, E], bf16, tag="win")
            identt = diagb[:, 0, :]

            # ---- input DMAs: each diag chunk split across BOTH HWDGE
            # rings so arrival paces the merged conv's consumption rate
            # (442KB per 1.6us chunk); small lead pieces cut the latency
            # to the first tap; one-hot halves late (needed at gathers)
            def diag_piece(eng, b0, b1):
                eng.dma_start(
                    out=diagb[:, b0:b1, :],
                    in_=diagw[:, b0 * 128 : b1 * 128].rearrange(
                        "p (k j) -> p k j", j=128
                    ),
                )

            CW = BPC * PPAD  # tab2 cols per chunk

            def t2_piece(eng, c0, c1):
                eng.dma_start(
                    out=t2m[:, c0:c1, :, :].rearrange("p c b q -> p (c b q)"),
                    in_=tab2[:, c0 * CW : c1 * CW],
                )

            def ohh_piece(eng, b):
                eng.dma_start(
                    out=oht[:, b, :], in_=ohh[:, b * S : (b + 1) * S]
                )

            diag_piece(nc.sync, 0, 7)  # identity + chunk-0 taps 0-5
            t2_piece(nc.scalar, 0, 2)
            diag_piece(nc.sync, 12, 23)  # chunk 1
            diag_piece(nc.scalar, 7, 12)  # chunk-0 taps 6-10
            diag_piece(nc.sync, 34, 45)  # chunk 3
            diag_piece(nc.scalar, 23, 34)  # chunk 2
            diag_piece(nc.sync, 56, 67)  # chunk 5
            t2_piece(nc.scalar, 2, 6)
            ohh_piece(nc.sync, 0)
            diag_piece(nc.scalar, 45, 56)  # chunk 4
            ohh_piece(nc.scalar, 1)

            dr = [0]
            dengines = (nc.vector.tensor_copy, nc.scalar.copy)

            def drain(dst, src):
                dengines[dr[0] % 2](dst, src)
                dr[0] += 1

            cts = {}

            def conv_taps(c):
                # conv in [e,p]: stationary diag block, both batches streamed
                pT = psumt.tile([128, BPC, P], f32, tag="pT", name="pT")
                for k in range(W):
                    nc.tensor.matmul(
                        out=pT[:, :, :],
                        lhsT=diagb[:, 1 + c * W + k, :],
                        rhs=t2m[:, c, :, k : k + P],
                        start=(k == 0),
                        stop=(k == W - 1),
                    )
                ct = ctp.tile([128, BPC, P], bf16, tag="ct", name="ct")
                drain(ct[:, :, :], pT[:, :, :])
                cts[c] = ct

            def conv_tp(c):
                # both windows of both batches into one paired bf16 PSUM
                # tile, drained immediately into the window tile
                cw = psumw.tile([128, BPC * 2, 128], bf16, tag="cw", name="cw")
                for b in range(BPC):
                    nc.tensor.transpose(
                        out=cw[:, b * 2, :],
                        in_=cts[c][:, b, 0:128],
                        identity=identt,
                    )
                    nc.tensor.transpose(
                        out=cw[:, b * 2 + 1, :],
                        in_=cts[c][:, b, CB_BASE : CB_BASE + 128],
                        identity=identt,
                    )
                drain(
                    win[:, :, :, c * 128 : (c + 1) * 128],
                    cw[:, :, :].rearrange("p (b w) j -> p b w j", w=2),
                )

            # ---- conv: weave transposes one chunk behind the taps so the
            # PE never waits on a ct drain
            conv_taps(0)
            conv_taps(1)
            conv_tp(0)
            for c in range(2, EC):
                conv_taps(c)
                conv_tp(c - 1)
            conv_tp(EC - 1)

            # conv PSUM done: release while the first gathers run on psg1
            psumw.release()
            psumt.release()
            psg2 = tc.alloc_tile_pool(name="psum_g2", bufs=2, space="PSUM")
            gpools = [psg1, psg2]
            gi = [0]

            obcur = [None]

            def pdrain(dst, src):
                # gpsimd cannot read PSUM on TRN2: DVE/ACT alternate
                drain(dst, src)

            def gather_tile(b, t, last=False):
                # single-pass gather: out[j, e] = sum_p oh[p, j] * C[p, e]
                tw = LASTW if t == NTILES - 1 else 128
                cc = win[:, b, 0, :] if t < NT_A else win[:, b, 1, :]
                pool = gpools[gi[0] % 2]
                gi[0] += 1
                pso = pool.tile([128, E], f32, tag="po", name="pso")
                for n0, nw in ((0, 512), (512, 256)):
                    nc.tensor.matmul(
                        out=pso[0:tw, n0 : n0 + nw],
                        lhsT=oht[:, b, t * 128 : t * 128 + tw],
                        rhs=cc[:, n0 : n0 + nw],
                        start=True,
                        stop=True,
                    )
                if t % 2 == 0:
                    obcur[0] = obp.tile([128, 2, E], bf16, tag="ob", name="ob2")
                ob2 = obcur[0]
                r0 = b * S + t * 128
                if last:
                    # final tile: drain halves on both engines, DMA halves on
                    # both HWDGE rings so the completions overlap
                    nc.vector.tensor_copy(ob2[0:tw, 0, 0:384], pso[0:tw, 0:384])
                    nc.scalar.copy(ob2[0:tw, 0, 384:768], pso[0:tw, 384:768])
                    nc.sync.dma_start(
                        out=out[r0 : r0 + tw, 0:384], in_=ob2[0:tw, 0, 0:384]
                    )
                    nc.scalar.dma_start(
                        out=out[r0 : r0 + tw, 384:768],
                        in_=ob2[0:tw, 0, 384:768],
                    )
                    return
                pdrain(ob2[0:tw, t % 2, :], pso[0:tw, :])
                if t % 2 == 1:
                    nc.sync.dma_start(
                        out=out[r0 - 128 : r0 + 128, :].rearrange(
                            "(t p) e -> p t e", t=2
                        ),
                        in_=ob2[:, 0:2, :],
                    )
                elif t == NTILES - 1:
                    nc.sync.dma_start(
                        out=out[r0 : r0 + tw, :], in_=ob2[0:tw, 0, :]
                    )

            for b in range(BPC):
                for t in range(NTILES):
                    gather_tile(b, t, last=(b == BPC - 1 and t == NTILES - 1))
            psg2.release()
            psg1.release()

    nc.finalize()
    return nc


def _get_nc():
    if "nc" not in _cache:
        _cache["nc"] = _build()
    return _cache["nc"]


def _prep_shared(data, w):
    # layout-only host staging (no arithmetic)
    import ml_dtypes

    bf = ml_dtypes.bfloat16
    d0 = np.asarray(data, dtype=np.float32)[:, :, 0, :]  # [100, 166, 768]
    # clip-pad positions to [176]
    dp = np.concatenate(
        [np.repeat(d0[:, :1], 5, axis=1), d0, np.repeat(d0[:, -1:], 5, axis=1)],
        axis=1,
    )  # [100, 176, 768]
    dT = np.transpose(dp, (0, 2, 1))  # [100, 768, 176]
    dT = dT.reshape(NSNIP, EC, 128, PPAD).transpose(0, 2, 1, 3)
    tabs = np.ascontiguousarray(dT.astype(bf))  # [100, 128, EC, PPAD]

    wT = np.asarray(w, dtype=np.float32).T  # [768, 11]
    w2 = wT.reshape(EC, 128, W).transpose(1, 0, 2).reshape(128, NBLK)
    diagw = np.zeros((128, NBLK + 1, 128), dtype=bf)
    ii = np.arange(128)
    diagw[ii, 0, ii] = 1  # block 0 = identity (for transpose matmuls)
    diagw[ii, 1:, ii] = w2.astype(bf)
    diagw = np.ascontiguousarray(diagw.reshape(128, (NBLK + 1) * 128))
    return tabs, diagw


def _prep_batch(idx_row):
    """Sort one batch's indices; return (one-hot [128, S] bf16, rank)."""
    import ml_dtypes

    v = np.asarray(idx_row, dtype=np.int64)
    order = np.argsort(v, kind="stable")
    vs = v[order]
    # sorted tiles 0..5 must fit CA rows [0,127]; tiles 6..8 CB rows [38,165]
    assert vs[NT_A * 128 - 1] <= 127, "gather tile/window layout violated (A)"
    assert vs[NT_A * 128] >= CB_BASE, "gather tile/window layout violated (B)"
    base = np.repeat([0] * NT_A + [CB_BASE] * (NTILES - NT_A), 128)[:S]
    loc = vs - base
    assert loc.min() >= 0 and loc.max() < 128
    oh = np.zeros((128, S), dtype=ml_dtypes.bfloat16)
    oh[loc, np.arange(S)] = 1
    rank = np.empty(S, dtype=np.int64)
    rank[order] = np.arange(S)
    return oh, rank


def kernel(inputs, code_snippet_id, data, w, _trace=False):
    from concourse.bass_utils import run_bass_kernel_spmd

    nc = _get_nc()
    inputs = np.asarray(inputs, dtype=np.int32)
    snips = np.asarray(code_snippet_id, dtype=np.int32).reshape(-1)
    tabs, diagw = _prep_shared(data, w)

    in_maps = []
    ranks = []
    for ci in range(N_CORES):
        b0 = ci * BPC
        ohs = []
        for b in range(BPC):
            oh, rank = _prep_batch(inputs[b0 + b])
            ohs.append(oh)
            ranks.append(rank)
        tb = np.stack([tabs[snips[b0 + b]] for b in range(BPC)], axis=2)
        in_maps.append(
            {
                "tab2": np.ascontiguousarray(tb.reshape(128, EC * BPC * PPAD)),
                "diagw": diagw,
                "ohh": np.ascontiguousarray(np.concatenate(ohs, axis=1)),
            }
        )

    res = run_bass_kernel_spmd(
        nc, in_maps, core_ids=list(range(N_CORES)), trace=_trace
    )
    _cache["last_results"] = res
    outs = []
    for ci in range(N_CORES):
        o = np.asarray(res.results[ci]["out"]).reshape(BPC, S, E)
        for b in range(BPC):
            outs.append(o[b, ranks[ci * BPC + b]].astype(np.float32))
    return np.stack(outs, axis=0)
